# revision 50
# baseline (speedup 1.0000x reference)
"""Trainium2 Bass kernel for the GravityODECell problem.

Physics per step (dt = 0.1 = DT/N_STEPS, 5 steps, 3 bodies in 2D per row):
    vec_i = p_i - p_{i+1 mod 3}
    u_i   = clip(|vec_i|^2, 1, 28900)
    w_i   = s * u_i^{-1.5}             (s folded per-variant, see below)
    Fs_i  = vec_i * w_i
    v    += Fs_{i-1} - Fs_i
    p    += dt * v

Sharding: pure data parallel over 8 NeuronCores (batch split), A replicated.

Implementation (per core, rows tiled [128 partitions x W rows x 6 comps]):
  - State p, q (= dt*v) kept in fp32 SBUF; force path in bf16.
  - Custom DVE uop GRAV_SS_CLIP computes clip(x^2+y^2, lo, hi) in ONE
    1-elem/cycle instruction (square+pairsum+clip fused).
  - w = exp(-1.5*ln(u) + ln|s|) on ScalarE (ln/exp share one table set);
    s = dt^2*A so the q-update needs no extra scaling.
  - dq_i = Fs_{i-1} - Fs_i computed on the TensorEngine as identity-matmul
    accumulations into PSUM (bf16 +/-I weights; exact for bf16 inputs).
  - q' = q + dq (DVE, PSUM operand); p' = p + q' (DVE/GPSIMD row-split).
  - w broadcast-expand (3 -> 6 comps) on GPSIMD.
"""

import functools
import math
import os
import sys

import numpy as np

for _p in ("/opt/trn_rl_repo", "/root/.axon_site/_ro/trn_rl_repo"):
    if os.path.isdir(_p) and _p not in sys.path:
        sys.path.insert(0, _p)

import concourse.bass as bass
import concourse.bacc as bacc
import concourse.mybir as mybir
from concourse.bass_utils import run_bass_kernel_spmd
from concourse.tile import TileContext

# --- activation-table chooser fix ------------------------------------------
# bacc's insert_act_table_loads picks the FIRST table set containing each
# activation function: Exp -> "exp_and_others" (id 0), Ln -> "natural_log"
# (id 5), so a Ln/Exp pair per step reloads tables twice per step (~2.7us
# each). Restrict the chooser's view so every function that
# "natural_log_exp_and_others" serves resolves to that one set. This only
# narrows the chooser's options (the chosen set genuinely contains the
# functions); runtime table contents and set ids are untouched.
import concourse.hw_specs as _hw_specs_mod

_orig_get_act_tables = _hw_specs_mod.get_activation_tables


@functools.lru_cache(maxsize=None)
def _patched_act_tables(arch):
    tabs = dict(_orig_get_act_tables(arch))
    keep = "natural_log_exp_and_others"
    if keep not in tabs:
        return tabs
    shared = tabs[keep]
    return {
        name: (fns if name == keep else fns - shared) for name, fns in tabs.items()
    }


_hw_specs_mod.get_activation_tables = _patched_act_tables
bacc.get_activation_tables = _patched_act_tables

N_CORES = 8
P = 128
W = int(os.environ.get("GRAV_W", "256"))   # rows per partition per tile
TILE_ROWS = P * W
DT_STEP = 0.1                # DT / N_STEPS = 0.5 / 5
N_STEPS = 5
SS_LO = 1.0
SS_HI = 28900.0              # 170^2
_S_SCALE = DT_STEP * DT_STEP  # w folds dt^2*A (q = dt*v state form)

F32 = mybir.dt.float32
BF16 = mybir.dt.bfloat16
ALU = mybir.AluOpType
ACTF = mybir.ActivationFunctionType

# --- tunables (variant flags; overridable via env for A/B testing) ---------
_env = os.environ.get
USE_CUSTOM_OP = _env("GRAV_CUSTOM_OP", "1") == "1"
USE_PE_DQ = _env("GRAV_PE_DQ", "1") == "1"
USE_GPSIMD_WD = _env("GRAV_GP_WD", "0") == "1"
PB_MODE = _env("GRAV_PB", "act_id")  # act_id | act_copy | dve
# fraction of W rows of the p-update handled by GPSIMD (rest on DVE)
GP_P_FRAC = float(_env("GRAV_GP_P_FRAC", "0.3"))
GROUP = int(_env("GRAV_GROUP", "8"))       # tiles emitted interleaved per group
BUFS = int(_env("GRAV_BUFS", str(GROUP)))
PSUM_BUFS = int(_env("GRAV_PSUM_BUFS", "1"))
SCALES_ON_ACT = _env("GRAV_SCALES_ACT", "1") == "1"
EMIT_MODE = _env("GRAV_EMIT", "tile")  # tile | stage | wave
# fraction of W rows of the Fs multiply handled by GPSIMD (rest on DVE)
GP_FS_FRAC = float(_env("GRAV_GP_FS_FRAC", "1.0"))
# SBUF trim: u in bf16 and w aliased into pb's (dead by then) first half
TRIM = _env("GRAV_TRIM", "0") == "1"
# fraction of W rows of the vecs subtract handled by GPSIMD (rest on DVE)
GP_VECS_FRAC = float(_env("GRAV_GP_VECS_FRAC", "0.0"))
# emit q' as 3 per-PSUM-bank adds (pipelines with PE) instead of one add
Q_SPLIT = _env("GRAV_Q_SPLIT", "0") == "1"
# fraction of W rows whose w is pre-expanded on ACT so DVE multiplies Fs
# at 2x for those rows (GPSIMD keeps the remaining GP_FS_FRAC portion)
ACT_WD_FRAC = float(_env("GRAV_ACT_WD_FRAC", "0.0"))


# --- custom DVE op: out = clip(in0^2 + in1^2, s0, s1) -----------------------
@functools.lru_cache(maxsize=1)
def _get_ss_clip_op():
    import concourse.dve_ops as dve_ops
    from concourse.dve_spec import C0, C1, Spec, Src0, Src1, lower, maxx, minn, sq
    from concourse.dve_uop import DveOpSpec

    for o in dve_ops.OPS:
        if o.name == "GRAV_SS_CLIP":
            return o

    def _ref(in0, in1, s0, s1, imm2):
        ss = in0.astype(np.float32) ** 2 + in1.astype(np.float32) ** 2
        return np.clip(ss, s0, s1).astype(np.float32)

    spec = Spec(body=minn(maxx(sq(Src0) + sq(Src1), C0), C1), reference=_ref)
    shas = {}
    for ver in ("v3", "v4"):
        try:
            shas[ver] = DveOpSpec(name="GRAV_SS_CLIP", uops=lower(spec, ver=ver)).sha(
                ver
            )
        except Exception:
            pass
    op = dve_ops.DveOp("GRAV_SS_CLIP", spec, subdim=False, uops_sha=shas)
    dve_ops.OPS.append(op)
    dve_ops.CUSTOM_DVE_SPECS[op.name] = spec
    dve_ops._SUB_OPCODE_FOR_NAME[op.name] = (
        dve_ops._CUSTOM_DVE_ROW_BASE + len(dve_ops.OPS) - 1
    )
    return op


@functools.lru_cache(maxsize=None)
def _build(b_core: int, s: float):
    """Per-core Bass kernel for b_core rows. s = DT_STEP^2 * A is baked in
    (w = |s| * u^-1.5 via the Exp bias; A's sign flips the +/-I weights)."""
    n_tiles = b_core // TILE_ROWS
    assert n_tiles * TILE_ROWS == b_core
    neg_s = s < 0.0
    ln_s = float(np.log(abs(s)))
    ss_op = _get_ss_clip_op() if USE_CUSTOM_OP else None

    nc = bacc.Bacc()

    # SBUF-resident [128,1] constant with ln|s| (bias operand of Exp).
    lnb_t = nc.alloc_sbuf_tensor("const-lnb", [P, 1], F32)
    nc.gpsimd.memset(lnb_t.ap(), ln_s)
    nc.const_aps.aps[(F32, ln_s)] = lnb_t.ap()

    if USE_PE_DQ:
        # Identity / -identity bf16 weight matrices built on-chip:
        # col[p, f] = f (iota along free), row-scalar [P,1] = p, then
        # I = (col == row) and mI = -(col == row).
        iota_c = nc.alloc_sbuf_tensor("iota-col", [P, P], F32)
        iota_r = nc.alloc_sbuf_tensor("iota-row", [P, 1], F32)
        ident_p = nc.alloc_sbuf_tensor("ident-pos", [P, P], BF16)
        ident_n = nc.alloc_sbuf_tensor("ident-neg", [P, P], BF16)
        nc.gpsimd.iota(
            iota_c.ap(),
            pattern=[[1, P]],
            base=0,
            channel_multiplier=0,
            allow_small_or_imprecise_dtypes=True,
        )
        nc.gpsimd.iota(
            iota_r.ap(),
            pattern=[[1, 1]],
            base=0,
            channel_multiplier=1,
            allow_small_or_imprecise_dtypes=True,
        )
        nc.all_engine_barrier()
        nc.vector.tensor_scalar(
            ident_p.ap(), iota_c.ap(), iota_r.ap(), None, ALU.is_equal
        )
        nc.vector.tensor_scalar(
            ident_n.ap(), iota_c.ap(), iota_r.ap(), -1.0, ALU.is_equal, ALU.mult
        )
        w_plus = ident_n.ap() if neg_s else ident_p.ap()
        w_minus = ident_p.ap() if neg_s else ident_n.ap()

    nc.all_engine_barrier()

    poss_in = nc.declare_dram_parameter("poss", [b_core, 6], F32, isOutput=False)
    vels_in = nc.declare_dram_parameter("vels", [b_core, 6], F32, isOutput=False)
    poss_out = nc.declare_dram_parameter("poss_out", [b_core, 6], F32, isOutput=True)
    vels_out = nc.declare_dram_parameter("vels_out", [b_core, 6], F32, isOutput=True)

    pr_in = poss_in.rearrange("(t p w) c -> t p (w c)", t=n_tiles, p=P, w=W)
    vr_in = vels_in.rearrange("(t p w) c -> t p (w c)", t=n_tiles, p=P, w=W)
    pr_out = poss_out.rearrange("(t p w) c -> t p (w c)", t=n_tiles, p=P, w=W)
    vr_out = vels_out.rearrange("(t p w) c -> t p (w c)", t=n_tiles, p=P, w=W)

    w_gp = int(round(W * GP_P_FRAC)) if GP_P_FRAC > 0 else 0
    w_gp = min(max(w_gp, 0), W)
    w_fs = int(round(W * GP_FS_FRAC)) if GP_FS_FRAC > 0 else 0
    w_fs = min(max(w_fs, 0), W)
    w_vc = int(round(W * GP_VECS_FRAC)) if GP_VECS_FRAC > 0 else 0
    w_vc = min(max(w_vc, 0), W)
    w_aw = int(round(W * ACT_WD_FRAC)) if ACT_WD_FRAC > 0 else 0
    w_aw = min(max(w_aw, 0), W)

    with TileContext(nc) as tc:
        with (
            tc.tile_pool(name="state", bufs=BUFS) as spool,
            tc.tile_pool(name="tmp", bufs=BUFS) as tpool,
            tc.tile_pool(name="psum", bufs=PSUM_BUFS, space="PSUM") as ppool,
        ):
            def alloc_ctx(t):
                ctx = {"t": t}
                tp = spool.tile([P, W, 6], F32, tag="p")
                tq = spool.tile([P, W, 6], F32, tag="q")
                pb = tpool.tile([P, W, 6], BF16, tag="pb")
                vecs = tpool.tile([P, W, 6], BF16, tag="vecs")
                u = tpool.tile([P, W, 3], BF16 if TRIM else F32, tag="u")
                if TRIM:
                    wv = pb  # w written into pb[:, :, 0:3]; pb dead after vecs
                else:
                    wv = tpool.tile([P, W, 3], BF16, tag="wv")
                fs = tpool.tile([P, W, 6], BF16, tag="fs")
                if USE_GPSIMD_WD:
                    wd = tpool.tile([P, W, 6], BF16, tag="wd")
                elif w_aw > 0:
                    wd = tpool.tile([P, w_aw, 6], BF16, tag="wd")
                else:
                    wd = None
                ctx["tp"], ctx["tq"], ctx["pb"] = tp, tq, pb
                ctx["vecs"], ctx["u"], ctx["wv"], ctx["fs"], ctx["wd"] = (
                    vecs, u, wv, fs, wd,
                )
                ctx["tp_f"] = tp[:, :, :].rearrange("p w c -> p (w c)")
                ctx["tq_f"] = tq[:, :, :].rearrange("p w c -> p (w c)")
                ctx["pb_f"] = pb[:, :, :].rearrange("p w c -> p (w c)")
                ctx["u_f"] = u[:, :, :].rearrange("p w c -> p (w c)")
                if TRIM:
                    ctx["wv_f"] = wv[:, :, 0:3]  # strided; pair with u3 view
                else:
                    ctx["wv_f"] = wv[:, :, :].rearrange("p w c -> p (w c)")
                ctx["u3"] = u[:, :, :]
                ctx["vecs4"] = vecs[:, :, :].rearrange("p w (i two) -> p w i two", two=2)
                ctx["fs4"] = fs[:, :, :].rearrange("p w (i two) -> p w i two", two=2)
                if TRIM:
                    ctx["wb"] = (
                        wv[:, :, 0:3].unsqueeze(3).broadcast_to((P, W, 3, 2))
                    )
                else:
                    ctx["wb"] = (
                        wv[:, :, :].unsqueeze(3).broadcast_to((P, W, 3, 2))
                    )
                nc.sync.dma_start(out=ctx["tp_f"], in_=pr_in[t])
                nc.sync.dma_start(out=ctx["tq_f"], in_=vr_in[t])
                # q = dt * v
                if SCALES_ON_ACT:
                    nc.scalar.mul(ctx["tq_f"], ctx["tq_f"], DT_STEP)
                else:
                    nc.vector.tensor_scalar_mul(ctx["tq_f"], ctx["tq_f"], DT_STEP)
                return ctx

            def emit_pb(ctx):
                if PB_MODE == "act_id":
                    nc.scalar.add(ctx["pb_f"], ctx["tp_f"], 0.0)
                elif PB_MODE == "act_copy":
                    nc.scalar.copy(ctx["pb_f"], ctx["tp_f"])
                else:
                    nc.vector.tensor_copy(ctx["pb_f"], ctx["tp_f"])

            def emit_vecs(ctx):
                pb, vecs = ctx["pb"], ctx["vecs"]
                if w_vc > 0:
                    nc.gpsimd.tensor_tensor(
                        vecs[:, 0:w_vc, 0:4], pb[:, 0:w_vc, 0:4],
                        pb[:, 0:w_vc, 2:6], ALU.subtract,
                    )
                    nc.gpsimd.tensor_tensor(
                        vecs[:, 0:w_vc, 4:6], pb[:, 0:w_vc, 4:6],
                        pb[:, 0:w_vc, 0:2], ALU.subtract,
                    )
                    if w_vc < W:
                        nc.vector.tensor_sub(
                            vecs[:, w_vc:W, 0:4], pb[:, w_vc:W, 0:4], pb[:, w_vc:W, 2:6]
                        )
                        nc.vector.tensor_sub(
                            vecs[:, w_vc:W, 4:6], pb[:, w_vc:W, 4:6], pb[:, w_vc:W, 0:2]
                        )
                else:
                    nc.vector.tensor_sub(vecs[:, :, 0:4], pb[:, :, 0:4], pb[:, :, 2:6])
                    nc.vector.tensor_sub(vecs[:, :, 4:6], pb[:, :, 4:6], pb[:, :, 0:2])

            def emit_u(ctx):
                u, u_f, vecs4 = ctx["u"], ctx["u_f"], ctx["vecs4"]
                if ss_op is not None:
                    nc.vector._custom_dve(
                        ss_op,
                        out=u[:, :, :],
                        in0=vecs4[:, :, :, 0],
                        in1=vecs4[:, :, :, 1],
                        s0=SS_LO,
                        s1=SS_HI,
                    )
                else:
                    vecs = ctx["vecs"]
                    sq6 = tpool.tile([P, W, 6], BF16, tag="sq6")
                    sq4 = sq6[:, :, :].rearrange("p w (i two) -> p w i two", two=2)
                    nc.vector.tensor_mul(
                        sq6[:, :, :].rearrange("p w c -> p (w c)"),
                        vecs[:, :, :].rearrange("p w c -> p (w c)"),
                        vecs[:, :, :].rearrange("p w c -> p (w c)"),
                    )
                    nc.vector.tensor_add(u[:, :, :], sq4[:, :, :, 0], sq4[:, :, :, 1])
                    nc.vector.tensor_scalar(u_f, u_f, SS_LO, SS_HI, ALU.max, ALU.min)

            def emit_ln(ctx):
                nc.scalar.activation(ctx["u_f"], ctx["u_f"], ACTF.Ln)

            def emit_exp(ctx):
                src = ctx["u3"] if TRIM else ctx["u_f"]
                nc.scalar.activation(
                    ctx["wv_f"], src, ACTF.Exp, bias=ln_s, scale=-1.5
                )

            def emit_wd(ctx):
                if USE_GPSIMD_WD:
                    wd = ctx["wd"]
                    wd4 = wd[:, :, :].rearrange("p w (i two) -> p w i two", two=2)
                    nc.gpsimd.tensor_copy(wd4[:, :, :, :], ctx["wb"])
                elif w_aw > 0:
                    # ACT expands w for the first w_aw rows so DVE's Fs
                    # multiply runs at 2x there (unit-stride bf16 operands)
                    wd = ctx["wd"]
                    wd4 = wd[:, :, :].rearrange("p w (i two) -> p w i two", two=2)
                    nc.scalar.copy(wd4[:, :, :, :], ctx["wb"][:, 0:w_aw, :, :])

            def emit_fs(ctx):
                vecs, fs, wd = ctx["vecs"], ctx["fs"], ctx["wd"]
                if USE_GPSIMD_WD:
                    nc.vector.tensor_mul(
                        fs[:, :, :].rearrange("p w c -> p (w c)"),
                        vecs[:, :, :].rearrange("p w c -> p (w c)"),
                        wd[:, :, :].rearrange("p w c -> p (w c)"),
                    )
                elif w_fs > 0:
                    # row-split: rows [0,w_aw) on DVE at 2x via the ACT-
                    # expanded wd; GPSIMD multiplies [w_aw,hi) with the
                    # broadcast w operand; DVE does any tail at 1x.
                    vecs4, fs4, wb = ctx["vecs4"], ctx["fs4"], ctx["wb"]
                    if w_aw > 0:
                        wd = ctx["wd"]
                        nc.vector.tensor_mul(
                            fs4[:, 0:w_aw, :, :].rearrange("p w i two -> p (w i two)"),
                            vecs4[:, 0:w_aw, :, :].rearrange("p w i two -> p (w i two)"),
                            wd[:, :, :].rearrange("p w c -> p (w c)"),
                        )
                    hi = min(max(w_fs, w_aw), W)
                    if hi > w_aw:
                        nc.gpsimd.tensor_tensor(
                            fs4[:, w_aw:hi, :, :],
                            vecs4[:, w_aw:hi, :, :],
                            wb[:, w_aw:hi, :, :],
                            ALU.mult,
                        )
                    if hi < W:
                        nc.vector.tensor_mul(
                            fs4[:, hi:W, :, :],
                            vecs4[:, hi:W, :, :],
                            wb[:, hi:W, :, :],
                        )
                else:
                    nc.vector.tensor_mul(
                        ctx["fs4"][:, :, :, :], ctx["vecs4"][:, :, :, :], ctx["wb"]
                    )

            def emit_update(ctx, step):
                tp, tq, fs = ctx["tp"], ctx["tq"], ctx["fs"]
                tq_f = ctx["tq_f"]
                if USE_PE_DQ:
                    # dq_i = Fs_{i-1} - Fs_i via identity matmuls into PSUM.
                    # PSUM layout [3 banks][W, 2]: bank c holds comps 2c:2c+2.
                    dq = ppool.tile([P, 3, W, 2], F32, tag="dq")
                    for c in range(3):
                        cm = (c + 2) % 3  # source edge i-1 comp block
                        nc.tensor.matmul(
                            dq[:, c, :, :],
                            w_plus,
                            fs[:, :, 2 * cm : 2 * cm + 2],
                            start=True,
                            stop=False,
                        )
                        nc.tensor.matmul(
                            dq[:, c, :, :],
                            w_minus,
                            fs[:, :, 2 * c : 2 * c + 2],
                            start=False,
                            stop=True,
                        )
                    tq4 = tq[:, :, :].rearrange("p w (c i) -> p w c i", i=2)
                    if Q_SPLIT:
                        # per-bank adds: Tile tracks PSUM deps at bank
                        # granularity, so q'_c can start once bank c's two
                        # matmuls finish while PE fills the other banks.
                        for c in range(3):
                            nc.vector.tensor_add(
                                tq4[:, :, c, :], tq4[:, :, c, :], dq[:, c, :, :]
                            )
                    else:
                        dq_q = dq[:, :, :, :].rearrange("p c w i -> p w c i")
                        nc.vector.tensor_add(tq4, tq4, dq_q)
                else:
                    dfs = tpool.tile([P, W, 6], BF16, tag="dfs")
                    nc.vector.tensor_sub(dfs[:, :, 0:2], fs[:, :, 4:6], fs[:, :, 0:2])
                    nc.vector.tensor_sub(dfs[:, :, 2:6], fs[:, :, 0:4], fs[:, :, 2:6])
                    dfs_f = dfs[:, :, :].rearrange("p w c -> p (w c)")
                    nc.vector.tensor_add(tq_f, tq_f, dfs_f)

                # p += q'   (row-split DVE / GPSIMD)
                if w_gp > 0:
                    nc.gpsimd.tensor_tensor(
                        tp[:, 0:w_gp, :], tp[:, 0:w_gp, :], tq[:, 0:w_gp, :], ALU.add
                    )
                if w_gp < W:
                    nc.vector.tensor_add(
                        tp[:, w_gp:W, :], tp[:, w_gp:W, :], tq[:, w_gp:W, :]
                    )

            def emit_store(ctx):
                t = ctx["t"]
                nc.sync.dma_start(out=pr_out[t], in_=ctx["tp_f"])
                # v_out = q / dt
                if SCALES_ON_ACT:
                    nc.scalar.mul(ctx["tq_f"], ctx["tq_f"], 1.0 / DT_STEP)
                else:
                    nc.vector.tensor_scalar_mul(ctx["tq_f"], ctx["tq_f"], 1.0 / DT_STEP)
                nc.sync.dma_start(out=vr_out[t], in_=ctx["tq_f"])

            # Interleaved, stage-major emission: GROUP tiles advance
            # step-by-step together, and within a step each pipeline stage is
            # emitted for every tile before the next stage. Per-engine streams
            # execute in order, so stage-major order keeps every engine fed
            # with independent work instead of stalling on the next tile's
            # cross-engine dependency. The PE+update tail stays per-tile so
            # only PSUM_BUFS dq slots are ever live.
            all_stages = (
                emit_pb, emit_vecs, emit_u, emit_ln, emit_exp, emit_wd, emit_fs
            )

            def emit_tile_step(ctx, step):
                for stage in all_stages:
                    stage(ctx)
                emit_update(ctx, step)

            if EMIT_MODE == "wave":
                # Rolling wavefront: admit one tile per round, advance every
                # in-flight tile one step, retire finished tiles. In-flight
                # tiles sit at staggered steps, so the pipeline never drains
                # at a group boundary.
                window = []  # [ctx, next_step]
                next_t = 0
                while window or next_t < n_tiles:
                    if next_t < n_tiles and len(window) < GROUP:
                        window.append([alloc_ctx(next_t), 0])
                        next_t += 1
                    for item in list(window):
                        ctx, s = item
                        emit_tile_step(ctx, s)
                        item[1] = s + 1
                        if item[1] == N_STEPS:
                            emit_store(ctx)
                            window.remove(item)
            else:
                for t0 in range(0, n_tiles, GROUP):
                    ctxs = [alloc_ctx(t) for t in range(t0, min(t0 + GROUP, n_tiles))]
                    for step in range(N_STEPS):
                        if EMIT_MODE == "stage":
                            for stage in all_stages:
                                for ctx in ctxs:
                                    stage(ctx)
                            for ctx in ctxs:
                                emit_update(ctx, step)
                        else:
                            for ctx in ctxs:
                                emit_tile_step(ctx, step)
                    for ctx in ctxs:
                        emit_store(ctx)

    nc.finalize()
    return nc


def _numpy_reference(poss, vels, A):
    p = poss.astype(np.float32).copy()
    v = vels.astype(np.float32).copy()
    A = np.float32(A)
    for _ in range(N_STEPS):
        b = p.reshape(-1, 3, 2)
        vecs = b - np.roll(b, -1, axis=1)
        ss = np.clip((vecs**2).sum(-1, keepdims=True), 0.1, 100000.0)
        norms = np.sqrt(ss)
        F = vecs / np.clip(norms, 1.0, 170.0) ** 3
        F = -(A * (F - np.roll(F, 1, axis=1)))
        v = v + np.float32(DT_STEP) * F.reshape(-1, 6)
        p = p + np.float32(DT_STEP) * v
    return p, v


def kernel(poss, vels, A):
    poss = np.ascontiguousarray(poss, dtype=np.float32)
    vels = np.ascontiguousarray(vels, dtype=np.float32)
    a_val = float(np.asarray(A))
    s = DT_STEP * DT_STEP * a_val

    b_total = poss.shape[0]
    if s == 0.0 or b_total % (N_CORES * TILE_ROWS) != 0:
        return _numpy_reference(poss, vels, a_val)

    b_core = b_total // N_CORES
    nc = _build(b_core, s)

    in_maps = [
        {
            "poss": poss[i * b_core : (i + 1) * b_core],
            "vels": vels[i * b_core : (i + 1) * b_core],
        }
        for i in range(N_CORES)
    ]
    res = run_bass_kernel_spmd(nc, in_maps, list(range(N_CORES)))
    poss_o = np.concatenate([r["poss_out"] for r in res.results], axis=0)
    vels_o = np.concatenate([r["vels_out"] for r in res.results], axis=0)
    return poss_o, vels_o
